# revision 2
# baseline (speedup 1.0000x reference)
"""Trainium2 Bass kernel for nn_CrossAttention_36309653521078 (v2).

Math notes:
  - seq_len == 1 => softmax over one key == 1, so attn == V exactly; Q/K are
    dead code.
  - attn == V also lets Wo fold into Wv on the host:
        Wc  = Wv_flat @ Wo            [3072, 512]
        bc  = bv_flat @ Wo + bo       [512]
        x1  = h_s @ Wc + (h_g + bc)   (hg residual carries the bias)
        ha  = LN1(x1); mlp = gelu(ha @ W1 + b1) @ W2 + b2
        out = LN2(mlp + ha)
  - All matmul operands are bf16 (same 1 cycle/row rate as f32r on TRN2,
    half the DMA/SBUF bytes). Empirical rel-err of the full bf16 chain vs
    the fp32 reference is 2.6e-3, well under the 2e-2 gate.
  - Feature-major layout ([D, B] activations); LN partition reductions use
    an all-ones [128,128] stationary matmul (result arrives broadcast).
  - Two-batch-tile software pipeline: tile t's Wc-matmul stream covers tile
    t-1's MLP matmuls and both tiles' LN stat/normalize chains, keeping the
    PE queue free of serial-chain bubbles.
Sharding: pure data parallelism over the batch dim across 8 cores.
"""

import numpy as np
import ml_dtypes

import concourse.bass as bass
import concourse.mybir as mybir
import concourse.tile as tile
from concourse.bass_utils import run_bass_kernel_spmd

F32 = mybir.dt.float32
BF16 = mybir.dt.bfloat16
NP_BF16 = ml_dtypes.bfloat16

N_CORES = 8
B = 16384
G_DIM = 512
S_DIM = 3072
HID = 512
H2 = 1024
BL = B // N_CORES      # rows per core
NB = 512               # batch-tile (moving free dim)
NBT = BL // NB         # batch tiles per core
EPS = 1e-5

SK = S_DIM // 128      # 24 k-subtiles for the fused Wc matmul
SKH = SK // 4          # k-subtiles per streamed hs chunk (4 chunks/tile)
KO = HID // 128        # 4
MH = H2 // 128         # 8

# consts tile column layout (each entry is [128, n] chunks of a vector)
_C_B1 = 0              # b1               [1024] -> cols 0:8
_C_B2 = 8              # b2               [512]  -> cols 8:12
_C_L1G = 12            # ln1_g            cols 12:16
_C_L1B = 16            # ln1_b            cols 16:20
_C_L2G = 20            # ln2_g            cols 20:24
_C_L2B = 24            # ln2_b            cols 24:28
_C_N = 28


def _split_multi_waits(nc):
    """The walrus build here rejects >1 sync-wait on several instruction
    codegen structs (Drain/CTRL, fused-LDW matmul). Hoist extra waits onto
    single-wait NOPs inserted just before the owning instruction."""
    for blk in nc.m.functions[0].blocks:
        insts = list(blk.instructions)
        out, changed, k = [], False, 0
        for inst in insts:
            si = inst.sync_info
            waits = list(si.on_wait) if si and si.on_wait else []
            if len(waits) > 1:
                for w in waits[:-1]:
                    out.append(mybir.InstNoOp(
                        name=f"wsplit-{blk.name}-{k}",
                        engine=inst.engine,
                        bass_nofuse=True,
                        sync_info=mybir.SyncInfo(on_wait=[w], on_update=[]),
                    ))
                    k += 1
                si.on_wait = [waits[-1]]
                changed = True
            out.append(inst)
        if changed:
            blk.instructions = out


def build_nc(reps: int = 1, split_waits: bool = True, timing: bool = False):
    """reps>1 repeats the whole per-core body (for differential timing).
    timing=True shrinks the DRAM activations (one batch-tile, re-read for
    every batch-tile) and adds a reps-sized marker output so that timing
    variants can't collide in any executable cache."""
    nc = bass.Bass("TRN2", target_bir_lowering=False, debug=False)

    bl = NB if timing else BL
    n_ht = 1 if timing else NBT
    hsT = nc.dram_tensor("hsT", [n_ht, 128, SK, NB], BF16,
                         kind="ExternalInput").ap()
    hgT = nc.dram_tensor("hgT", [HID, bl], F32, kind="ExternalInput").ap()
    wc = nc.dram_tensor("wc", [128, SK, HID], BF16, kind="ExternalInput").ap()
    w1 = nc.dram_tensor("w1", [128, KO, H2], BF16, kind="ExternalInput").ap()
    w2 = nc.dram_tensor("w2", [128, MH, HID], BF16, kind="ExternalInput").ap()
    cst = nc.dram_tensor("cst", [128, _C_N], F32, kind="ExternalInput").ap()
    outT = nc.dram_tensor("outT", [HID, bl], BF16, kind="ExternalOutput").ap()
    mark = None
    if timing:
        mark = nc.dram_tensor("mark", [1, 8 * reps], F32,
                              kind="ExternalOutput").ap()

    hgT_t = hgT.rearrange("(c p) b -> p c b", p=128)
    outT_t = outT.rearrange("(c p) b -> p c b", p=128)

    with tile.TileContext(nc) as tc:
        with (
            nc.allow_low_precision(
                reason="bf16 matmul path: 2.6e-3 rel err vs 2e-2 budget"),
            tc.tile_pool(name="weights", bufs=1) as wpool,
            tc.tile_pool(name="hs", bufs=8) as hs_pool,
            tc.tile_pool(name="hg", bufs=2) as hg_pool,
            tc.tile_pool(name="x1", bufs=2) as x1_pool,
            tc.tile_pool(name="xsq", bufs=8) as xsq_pool,
            tc.tile_pool(name="g", bufs=8) as g_pool,
            tc.tile_pool(name="stat", bufs=3) as stat_pool,
            tc.tile_pool(name="x2", bufs=2) as x2_pool,
            tc.tile_pool(name="psum", bufs=8, space="PSUM") as psum,
        ):
            # ---- startup: hs(0)/hg(0) first, then weights (chunked) ----
            def dma_hs(t):
                tiles = []
                for c in range(4):
                    ht = hs_pool.tile([128, SKH, NB], BF16, tag="hs",
                                      name=f"hs{t}_{c}")
                    nc.sync.dma_start(
                        out=ht,
                        in_=hsT[t % n_ht, :, c * SKH:(c + 1) * SKH, :])
                    tiles.append(ht)
                return tiles

            def dma_hg(t, bsl):
                hg_t = hg_pool.tile([128, KO, NB], F32, tag="hg",
                                    name=f"hg{t}")
                nc.sync.dma_start(out=hg_t, in_=hgT_t[:, :, bsl])
                return hg_t

            wc_sb = wpool.tile([128, SK, HID], BF16)

            def dma_wc(a, b):
                nc.sync.dma_start(out=wc_sb[:, a:b, :], in_=wc[:, a:b, :])

            def dma_hs_chunk(t, c):
                ht = hs_pool.tile([128, SKH, NB], BF16, tag="hs",
                                  name=f"hs{t}_{c}")
                nc.sync.dma_start(
                    out=ht, in_=hsT[t % n_ht, :, c * SKH:(c + 1) * SKH, :])
                return ht

            # first-needed-first: interleave wc / hs(0) chunks in the order
            # the Wc matmul stream consumes them; hg(0) is needed last.
            dma_wc(0, 3)
            hs0 = [dma_hs_chunk(0, 0)]
            dma_wc(3, 6)
            hs0.append(dma_hs_chunk(0, 1))
            dma_wc(6, 12)
            hs0.append(dma_hs_chunk(0, 2))
            dma_wc(12, 18)
            hs0.append(dma_hs_chunk(0, 3))
            dma_wc(18, 24)
            hs_tiles = {0: hs0}
            first_bsl = slice(0, NB)
            hg_tiles = {0: dma_hg(0, first_bsl)}
            w1_sb = wpool.tile([128, KO, H2], BF16)
            nc.sync.dma_start(out=w1_sb, in_=w1)
            w2_sb = wpool.tile([128, MH, HID], BF16)
            nc.sync.dma_start(out=w2_sb, in_=w2)
            consts = wpool.tile([128, _C_N], F32)
            nc.sync.dma_start(out=consts, in_=cst)
            ones_f = wpool.tile([128, 128], F32)
            nc.vector.memset(ones_f, 1.0)
            ones128 = wpool.tile([128, 128], BF16)
            nc.scalar.activation(ones128, ones_f,
                                 mybir.ActivationFunctionType.Copy)
            eps_col = wpool.tile([128, 1], F32)
            nc.vector.memset(eps_col, EPS)
            mark_sb = None
            if timing:
                mark_sb = wpool.tile([1, 8], F32)
                nc.vector.memset(mark_sb, 1.0)

            inv_n = 1.0 / HID
            ACT = mybir.ActivationFunctionType

            def stats_mm(x_chunks, xsq_chunks, tag):
                """sum / sum-of-squares over features via ones-matmuls.
                Returns (sumB, sqB) psum tiles [128, NB] (broadcast rows)."""
                sumB = psum.tile([128, NB], F32, tag="psum_mm",
                                 name=f"sum_{tag}")
                for j in range(KO):
                    nc.tensor.matmul(sumB, ones128, x_chunks[j],
                                     start=(j == 0), stop=(j == KO - 1))
                sqB = psum.tile([128, NB], F32, tag="psum_mm",
                                name=f"sq_{tag}")
                for j in range(KO):
                    nc.tensor.matmul(sqB, ones128, xsq_chunks[j],
                                     start=(j == 0), stop=(j == KO - 1))
                return sumB, sqB

            def stats_chain(sumB, sqB, tag):
                """mu / rsqrt(var+eps) broadcast tiles from stat psums."""
                muB = stat_pool.tile([128, NB], F32, tag="muB",
                                     name=f"mu_{tag}")
                nc.scalar.activation(muB, sumB, ACT.Copy, scale=inv_n)
                rB = stat_pool.tile([128, NB], F32, tag="rB",
                                    name=f"r_{tag}")
                nc.scalar.activation(rB, sqB, ACT.Copy, scale=inv_n)
                musqB = stat_pool.tile([128, NB], F32, tag="musqB",
                                       name=f"musq_{tag}")
                nc.vector.tensor_mul(musqB, muB, muB)
                nc.vector.tensor_sub(rB, rB, musqB)
                nc.scalar.activation(rB, rB, ACT.Sqrt, bias=eps_col)
                nc.vector.reciprocal(rB, rB)
                return muB, rB

            def normalize(x, muB, rB, gcol, bcol):
                """x[:, j, :] = ((x - mu) * r) * g + beta, in place."""
                for j in range(KO):
                    nc.vector.tensor_sub(x[:, j, :], x[:, j, :], muB)
                    nc.vector.tensor_mul(x[:, j, :], x[:, j, :], rB)
                    nc.scalar.activation(
                        x[:, j, :], x[:, j, :], ACT.Identity,
                        bias=consts[:, bcol + j: bcol + j + 1],
                        scale=consts[:, gcol + j: gcol + j + 1],
                    )

            # per-tile state carried across pipeline stages
            st = {}

            def front(rep, t, bsl):
                """Wc matmul stream + x1 evac + LN1 stat inputs for tile t."""
                ht = hs_tiles.pop(t)
                psumA = [psum.tile([128, NB], F32, tag="psum_mm",
                                   name=f"pA{rep}_{t}_{m}")
                         for m in range(KO)]
                for kc in range(SK):
                    ch, kk = divmod(kc, SKH)
                    for m in range(KO):
                        nc.tensor.matmul(
                            psumA[m],
                            wc_sb[:, kc, m * 128:(m + 1) * 128],
                            ht[ch][:, kk, :],
                            start=(kc == 0), stop=(kc == SK - 1),
                        )
                hg_t = hg_tiles.pop(t)
                x1 = x1_pool.tile([128, KO, NB], BF16, tag="x1",
                                  name=f"x1_{rep}_{t}")
                xsq = []
                for m in range(KO):
                    nc.vector.tensor_add(x1[:, m, :], psumA[m],
                                         hg_t[:, m, :])
                    xq = xsq_pool.tile([128, NB], BF16, tag="xsq",
                                       name=f"xq1_{rep}_{t}_{m}")
                    nc.vector.tensor_mul(xq, x1[:, m, :], x1[:, m, :])
                    xsq.append(xq)
                st[t] = {"x1": x1, "xsq1": xsq}

            def mlp(rep, t):
                """W1 + gelu + W2 + x2 evac for tile t (x1 already LN'd)."""
                s = st[t]
                x1 = s["x1"]
                g_sb = []
                for m in range(MH):
                    p1 = psum.tile([128, NB], F32, tag="psum_mm",
                                   name=f"p1_{rep}_{t}_{m}")
                    for k in range(KO):
                        nc.tensor.matmul(
                            p1, w1_sb[:, k, m * 128:(m + 1) * 128],
                            x1[:, k, :],
                            start=(k == 0), stop=(k == KO - 1),
                        )
                    g = g_pool.tile([128, NB], BF16, tag="g",
                                    name=f"g{rep}_{t}_{m}")
                    nc.scalar.activation(g, p1, ACT.Gelu,
                                         bias=consts[:, _C_B1 + m:
                                                     _C_B1 + m + 1])
                    g_sb.append(g)
                psum2 = [psum.tile([128, NB], F32, tag="psum_mm",
                                   name=f"p2_{rep}_{t}_{m}")
                         for m in range(KO)]
                for k in range(MH):
                    for m in range(KO):
                        nc.tensor.matmul(
                            psum2[m], w2_sb[:, k, m * 128:(m + 1) * 128],
                            g_sb[k],
                            start=(k == 0), stop=(k == MH - 1),
                        )
                x2 = x2_pool.tile([128, KO, NB], BF16, tag="x2",
                                  name=f"x2_{rep}_{t}")
                xsq2 = []
                for m in range(KO):
                    nc.scalar.activation(x2[:, m, :], psum2[m], ACT.Identity,
                                         bias=consts[:, _C_B2 + m:
                                                     _C_B2 + m + 1])
                    nc.vector.tensor_add(x2[:, m, :], x2[:, m, :],
                                         x1[:, m, :])
                    xq = xsq_pool.tile([128, NB], BF16, tag="xsq",
                                       name=f"xq2_{rep}_{t}_{m}")
                    nc.vector.tensor_mul(xq, x2[:, m, :], x2[:, m, :])
                    xsq2.append(xq)
                s["x2"] = x2
                s["xsq2"] = xsq2

            def ln1_finish(rep, t):
                s = st[t]
                sumB, sqB = stats_mm(
                    [s["x1"][:, j, :] for j in range(KO)], s["xsq1"],
                    f"a{rep}_{t}")
                muB, rB = stats_chain(sumB, sqB, f"a{rep}_{t}")
                normalize(s["x1"], muB, rB, _C_L1G, _C_L1B)

            def ln2_finish(rep, t, bsl):
                s = st.pop(t)
                sumB, sqB = stats_mm(
                    [s["x2"][:, j, :] for j in range(KO)], s["xsq2"],
                    f"b{rep}_{t}")
                muB, rB = stats_chain(sumB, sqB, f"b{rep}_{t}")
                normalize(s["x2"], muB, rB, _C_L2G, _C_L2B)
                nc.sync.dma_start(out=outT_t[:, :, bsl], in_=s["x2"])

            def bsl_of(t):
                return slice(0, NB) if timing else slice(t * NB,
                                                         (t + 1) * NB)

            for rep in range(reps):
                for t in range(NBT + 1):
                    if t < NBT:
                        front(rep, t, bsl_of(t))
                        # prefetch next tile's activations
                        nt = t + 1
                        if nt < NBT:
                            hs_tiles[nt] = dma_hs(nt)
                            hg_tiles[nt] = dma_hg(nt, bsl_of(nt))
                        elif rep + 1 < reps:
                            hs_tiles[0] = dma_hs(0)
                            hg_tiles[0] = dma_hg(0, bsl_of(0))
                    if t < NBT:
                        ln1_finish(rep, t)
                    if t >= 1:
                        mlp(rep, t - 1)
                        ln2_finish(rep, t - 1, bsl_of(t - 1))
            if timing:
                for rep in range(reps):
                    nc.sync.dma_start(out=mark[0:1, 8 * rep: 8 * (rep + 1)],
                                      in_=mark_sb)

    if split_waits:
        _split_multi_waits(nc)
    return nc


def _chunk_cols(vec):
    """[n*128] -> [128, n] with column j = vec[j*128:(j+1)*128]."""
    return np.ascontiguousarray(vec.reshape(-1, 128).T.astype(np.float32))


def _make_consts(inputs):
    cst = np.concatenate(
        [
            _chunk_cols(np.asarray(inputs["b1"], np.float32)),
            _chunk_cols(np.asarray(inputs["b2"], np.float32)),
            _chunk_cols(np.asarray(inputs["ln1_g"], np.float32)),
            _chunk_cols(np.asarray(inputs["ln1_b"], np.float32)),
            _chunk_cols(np.asarray(inputs["ln2_g"], np.float32)),
            _chunk_cols(np.asarray(inputs["ln2_b"], np.float32)),
        ],
        axis=1,
    )
    assert cst.shape == (128, _C_N)
    return cst


def _wT_tiles(w, kdim, ndim):
    """[K, N] fp32 -> [128, K//128, N] bf16, per-partition contiguous."""
    a = np.asarray(w, np.float32).reshape(kdim // 128, 128, ndim)
    return np.ascontiguousarray(a.transpose(1, 0, 2).astype(NP_BF16))


def _shared_weights(inputs):
    Wv = np.asarray(inputs["Wv"], np.float32)
    Wo = np.asarray(inputs["Wo"], np.float32)
    Wc = Wv.transpose(1, 0, 2).reshape(S_DIM, HID) @ Wo
    return {
        "wc": _wT_tiles(Wc, S_DIM, HID),
        "w1": _wT_tiles(np.asarray(inputs["W1"], np.float32), HID, H2),
        "w2": _wT_tiles(np.asarray(inputs["W2"], np.float32), H2, HID),
        "cst": _make_consts(inputs),
    }


def _hs_tiles(hs_rows):
    """[bl, S_DIM] fp32 -> [NBT, 128, SK, NB] bf16 (tile-major, per-
    partition contiguous: arr[t, p, kc, b] = hs[t*NB+b, kc*128+p])."""
    bl = hs_rows.shape[0]
    a = hs_rows.astype(NP_BF16).T.reshape(SK, 128, bl // NB, NB)
    return np.ascontiguousarray(a.transpose(2, 1, 0, 3))


def _residual(inputs):
    """bc = bv_flat @ Wo + bo, folded into the h_g residual."""
    bv = np.asarray(inputs["bv"], np.float32).reshape(HID)
    Wo = np.asarray(inputs["Wo"], np.float32)
    return bv @ Wo + np.asarray(inputs["bo"], np.float32)


def _prepare_in_maps(inputs):
    h_g = np.asarray(inputs["h_g"], np.float32)
    h_s = np.asarray(inputs["h_s"], np.float32)
    bc = _residual(inputs)
    shared = _shared_weights(inputs)
    in_maps = []
    for c in range(N_CORES):
        rows = slice(c * BL, (c + 1) * BL)
        in_maps.append({
            "hsT": _hs_tiles(h_s[rows]),
            "hgT": np.ascontiguousarray(h_g[rows].T + bc[:, None]),
            **shared,
        })
    return in_maps


def _prepare_timing_in_maps(inputs):
    h_g = np.asarray(inputs["h_g"], np.float32)
    h_s = np.asarray(inputs["h_s"], np.float32)
    bc = _residual(inputs)
    shared = _shared_weights(inputs)
    m = {
        "hsT": _hs_tiles(h_s[:NB]),
        "hgT": np.ascontiguousarray(h_g[:NB].T + bc[:, None]),
        **shared,
    }
    return [dict(m) for _ in range(N_CORES)]


def _assemble(results):
    return np.ascontiguousarray(
        np.concatenate([r["outT"].astype(np.float32).T for r in results],
                       axis=0))


def run(inputs, trace=False):
    nc = build_nc()
    in_maps = _prepare_in_maps(inputs)
    res = run_bass_kernel_spmd(nc, in_maps, list(range(N_CORES)), trace=trace)
    return _assemble(res.results), res


def kernel(**inputs):
    out, _ = run(inputs, trace=False)
    return out
